# revision 47
# baseline (speedup 1.0000x reference)
"""Trainium2 Bass kernel for the attention module:

    s = einsum('bqd,bad->bqa', q, a)      # [B, Nq, Na]
    e = softmax(s, axis=1)                 # over the Nq axis
    e = e / sum(e, axis=1)                 # identity (col sums are 1)
    h = einsum('bqa,bqd->bad', e, q)       # [B, Na, D]

Strategy: pure data parallel over B across 8 NeuronCores (2 batches/core).

Host-side prep: q and a are cast to fp16 (same 11-bit mantissa class as
TF32/f32r, so gemm1 logit precision is unchanged) and pre-transposed into
DMA-perfect layouts, so the device does NO transposes and NO casts:
  qh_d[b, p, ik, d]   = q[b, ik*128+p, d]          (gemm2 moving operand)
  qT_d[b, h, p, k, i] = q[b, h*1024+i, k*128+p]    (gemm1 moving operand)
  aT_d[b, jt, p, k, j] = a[b, jt*128+j, k*128+p]   (gemm1 stationary)

Per j-tile (128 output rows), steady state:
  gemm1: s^T[j, i] = sum_k aT[k]^T @ qT[k]  into one [128, 2048] PSUM tile
         (half-outer so exp-lo can start at half time; strip-outer /
         k-inner so 8 consecutive MMs accumulate into one PSUM bank)
  softmax along the free axis: true max = max(lo-half, hi-half) so
  exp(s - max) <= 1 fits fp16 exactly; ACT emits fp16 + f32 row-sums.
  DMA xbar transpose flips eT [j, i] -> e_sb [i, ik, j] off the PE.
  gemm2: h[j, d] = sum_ik e_sb[ik]^T @ qh[ik]  (fp16, strip-outer so
         strip 0's 1/rowsum scale + store overlap strip 1's matmuls).

All matmuls are fp16 so weight loads are standalone InstLdweights
(hideable), unlike f32r's self-loading matmuls. The j-loop is software
pipelined ACROSS batches with a 2-deep gemm2 queue: gemm2(t-2) runs
between gemm1(t) and gemm1(t+1) even at batch boundaries, hiding the
softmax chain, the single-buffered [128,2048] s-PSUM reuse (4+2 banks
of 8 used), and the HBM-bandwidth-bound prologue's late qh arrival.
"""

import numpy as np

import concourse.bass as bass
import concourse.tile as tile
from concourse import bacc, mybir

f32 = mybir.dt.float32
fp16 = mybir.dt.float16
bf16 = mybir.dt.bfloat16
AX = mybir.AxisListType
ALU = mybir.AluOpType
ACTF = mybir.ActivationFunctionType

P = 128

B, NQ, NA, D = 16, 2048, 2048, 1024
NCORES = 8
BLOC = B // NCORES


def build(bloc=BLOC, nq=NQ, na=NA, d=D, reps=1, num_devices=NCORES,
          unroll=1):
    """Build the per-core Bass program. All sizes must be multiples of 128."""
    ni = nq // P            # i-tiles (q rows; gemm2 contraction)
    nj = na // P            # j-tiles (a rows / output rows)
    nd = d // P             # d-tiles (gemm1 contraction)
    s_q = 512               # gemm1 moving strip (psum-bank width)
    h_q = nq // 2           # half of the i axis

    nc = bacc.Bacc("TRN2", target_bir_lowering=False, debug=False,
                   num_devices=num_devices)
    qh_d = nc.dram_tensor("qh", [bloc, P, ni, d], fp16,
                          kind="ExternalInput").ap()
    qT_d = nc.dram_tensor("qT", [bloc, 2, P, nd, h_q], fp16,
                          kind="ExternalInput").ap()
    aT_d = nc.dram_tensor("aT", [bloc, nj, P, nd, P], fp16,
                          kind="ExternalInput").ap()
    h_d = nc.dram_tensor("h", [bloc, na, d], bf16, kind="ExternalOutput").ap()

    from contextlib import ExitStack

    with tile.TileContext(nc) as tc, ExitStack() as ctx:
        qtpool_lo = ctx.enter_context(tc.tile_pool(name="qtpool_lo", bufs=2))
        qtpool_hi = ctx.enter_context(tc.tile_pool(name="qtpool_hi", bufs=2))
        qhpool = ctx.enter_context(tc.tile_pool(name="qhpool", bufs=2))
        apool = ctx.enter_context(tc.tile_pool(name="apool", bufs=4))
        etpool = ctx.enter_context(tc.tile_pool(name="etpool", bufs=1))
        espool = ctx.enter_context(tc.tile_pool(name="espool", bufs=3))
        hpool = ctx.enter_context(tc.tile_pool(name="hpool", bufs=2))
        stat = ctx.enter_context(tc.tile_pool(name="stat", bufs=3))
        const = ctx.enter_context(tc.tile_pool(name="const", bufs=1))
        ps_s = ctx.enter_context(tc.tile_pool(name="ps_s", bufs=1, space="PSUM"))
        ps_h = ctx.enter_context(tc.tile_pool(name="ps_h", bufs=1, space="PSUM"))
        ps_h2 = ctx.enter_context(tc.tile_pool(name="ps_h2", bufs=1, space="PSUM"))
        ps_wu = ctx.enter_context(tc.tile_pool(name="ps_wu", bufs=1, space="PSUM"))

        q_pend = {}

        def start_q_loads(b, prologue=False):
            """Allocate + DMA batch b's q tensors (fp16, pre-laid-out)."""
            if b >= bloc:
                return
            qT_lo = qtpool_lo.tile([P, nd, h_q], fp16, name="qT_lo")
            qT_hi = qtpool_hi.tile([P, nd, h_q], fp16, name="qT_hi")
            qh = qhpool.tile([P, ni, d], fp16, name="qh")
            if prologue:
                # b=0: per-k slices on the scalar + gpsimd queues so
                # gemm1(0) starts as soon as the first k-slices of the lo
                # half land (the SP queue leads with the aT tiles).
                for k in range(nd):
                    eng = nc.scalar if k % 2 == 0 else nc.gpsimd
                    eng.dma_start(out=qT_lo[:, k, :], in_=qT_d[b, 0, :, k, :])
                # interleave hi-half slices with qh chunks: gemm2(0) needs
                # qh's first chunks not long after gemm1 needs the hi half.
                for k in range(nd):
                    eng = nc.scalar if k % 2 == 0 else nc.gpsimd
                    eng.dma_start(out=qT_hi[:, k, :], in_=qT_d[b, 1, :, k, :])
                    if k % 2 == 1:
                        ik = 2 * (k // 2)
                        nc.gpsimd.dma_start(out=qh[:, ik:ik + 2, :],
                                            in_=qh_d[b, :, ik:ik + 2, :])
                for ik in range(8, ni, 4):
                    nc.gpsimd.dma_start(out=qh[:, ik:ik + 4, :],
                                        in_=qh_d[b, :, ik:ik + 4, :])
            else:
                # prefetch for b+1 issued mid-loop of batch b: big strided
                # DMAs, split across the two non-critical queues.
                nc.gpsimd.dma_start(out=qT_lo[:], in_=qT_d[b, 0])
                nc.gpsimd.dma_start(out=qT_hi[:], in_=qT_d[b, 1])
                nc.gpsimd.dma_start(out=qh[:, 0:ni // 2, :],
                                    in_=qh_d[b, :, 0:ni // 2, :])
                nc.gpsimd.dma_start(out=qh[:, ni // 2:, :],
                                    in_=qh_d[b, :, ni // 2:, :])
            q_pend[b] = (qT_lo, qT_hi, qh)

        def a_prep(b, jt, split=False):
            aT = apool.tile([P, nd, P], fp16, name="aT")
            if split:
                # first k-slice alone: the program's first Ldweights needs
                # only aT[:, 0, :], so a small leading DMA shaves ~1us off
                # the cold-start gap.
                nc.sync.dma_start(out=aT[:, 0, :], in_=aT_d[b, jt, :, 0, :])
                nc.sync.dma_start(out=aT[:, 1:, :], in_=aT_d[b, jt, :, 1:, :])
            else:
                nc.sync.dma_start(out=aT[:], in_=aT_d[b, jt])
            return aT

        def gemm1(aT, qT_pair, ps, fill=None):
            # half-outer (exp-lo can start at half time), then STRIP-outer /
            # k-inner: 8 consecutive matmuls accumulate into the same PSUM
            # region. Alternating psum banks per-MM causes HAM re-throttle
            # micro-idles on hw (tensor-engine doc, K18 failure mode).
            # fill: during the DMA-paced prologue, pad between strips with
            # dependency-free dummy MMs so the DVFS/HAM streak never drops.
            for h, qTh in enumerate(qT_pair):
                for st in range(h_q // s_q):
                    lo = h * h_q + st * s_q
                    for k in range(nd):
                        nc.tensor.matmul(
                            ps[:, lo:lo + s_q],
                            aT[:, k, :],
                            qTh[:, k, st * s_q:(st + 1) * s_q],
                            start=(k == 0), stop=(k == nd - 1))
                    if fill is not None:
                        warmup(fill, 3)

        def stats(ps):
            # softmax over the free axis with the TRUE row max (lo/hi
            # combined), so exp(s - max) in (0, 1] is exact in fp16.
            nm_lo = stat.tile([P, 1], f32, name="nm_lo")
            nm_hi = stat.tile([P, 1], f32, name="nm_hi")
            nm = stat.tile([P, 1], f32, name="nm")
            nc.vector.tensor_reduce(nm_lo[:], ps[:, 0:h_q], axis=AX.X,
                                    op=ALU.max, negate=True)
            nc.vector.tensor_reduce(nm_hi[:], ps[:, h_q:], axis=AX.X,
                                    op=ALU.max, negate=True)
            nc.vector.tensor_tensor(nm[:], nm_lo[:], nm_hi[:], op=ALU.min)
            eT_lo = etpool.tile([P, h_q], fp16, name="eT_lo")
            eT_hi = etpool.tile([P, h_q], fp16, name="eT_hi")
            e_sb = espool.tile([P, ni, P], fp16, name="e_sb")
            S1 = stat.tile([P, 1], f32, name="S1")
            S2 = stat.tile([P, 1], f32, name="S2")
            nc.scalar.activation(eT_lo[:], ps[:, 0:h_q], ACTF.Exp,
                                 bias=nm[:], scale=1.0, accum_out=S1[:])
            nc.scalar.activation(eT_hi[:], ps[:, h_q:], ACTF.Exp,
                                 bias=nm[:], scale=1.0, accum_out=S2[:])
            h_i = ni // 2
            nc.scalar.dma_start_transpose(e_sb[:, 0:h_i, :], eT_lo[:])
            nc.scalar.dma_start_transpose(e_sb[:, h_i:, :], eT_hi[:])
            S = stat.tile([P, 1], f32, name="S")
            nc.vector.tensor_tensor(S[:], S1[:], S2[:], op=ALU.add)
            rS = stat.tile([P, 1], f32, name="rS")
            nc.vector.reciprocal(rS[:], S[:])
            return e_sb, rS

        def gemm2(b, jt, e_sb, rS, qh, tail=False, chunked=False):
            if tail:
                # st-outer / ik-inner, narrow strips: each strip finishes
                # fully then drains, so the exposed final scale+store is a
                # quarter-width tail. The last strips' stores go out on the
                # otherwise-done scalar + gpsimd queues so the final drain
                # barrier doesn't wait on a serialized SP store chain.
                s_d = d // 4
                h_sb = hpool.tile([P, d], bf16, name="h_sb")
                store_eng = [nc.sync, nc.sync, nc.sync, nc.sync]
                for st in range(4):
                    psum_h = (ps_h if st % 2 == 0 else ps_h2).tile(
                        [P, s_d], f32, name="psum_h")
                    for ik in range(ni):
                        nc.tensor.matmul(
                            psum_h[:],
                            e_sb[:, ik, :],
                            qh[:, ik, st * s_d:(st + 1) * s_d],
                            start=(ik == 0), stop=(ik == ni - 1))
                    nc.vector.tensor_scalar_mul(
                        h_sb[:, st * s_d:(st + 1) * s_d], psum_h[:], rS[:])
                    store_eng[st].dma_start(
                        out=h_d[b, jt * P:(jt + 1) * P,
                                st * s_d:(st + 1) * s_d],
                        in_=h_sb[:, st * s_d:(st + 1) * s_d])
                return
            s_d = d // 2
            h_sb = hpool.tile([P, d], bf16, name="h_sb")
            if chunked:
                # first tile of the batch-0 prologue: ik-outer so matmuls
                # consume qh chunks as their DMAs land instead of waiting
                # for the whole tensor.
                psums = [ps_h.tile([P, s_d], f32, name="psum_h"),
                         ps_h2.tile([P, s_d], f32, name="psum_h2")]
                for ik in range(ni):
                    for st in range(2):
                        nc.tensor.matmul(
                            psums[st][:],
                            e_sb[:, ik, :],
                            qh[:, ik, st * s_d:(st + 1) * s_d],
                            start=(ik == 0), stop=(ik == ni - 1))
                for st in range(2):
                    nc.vector.tensor_scalar_mul(
                        h_sb[:, st * s_d:(st + 1) * s_d], psums[st][:], rS[:])
                    nc.sync.dma_start(
                        out=h_d[b, jt * P:(jt + 1) * P,
                                st * s_d:(st + 1) * s_d],
                        in_=h_sb[:, st * s_d:(st + 1) * s_d])
                return
            # strip-outer / ik-inner: 16 consecutive matmuls per PSUM bank
            # (HAM-friendly); strip 0's scale+store overlaps strip 1's MMs.
            for st in range(2):
                psum_h = (ps_h if st == 0 else ps_h2).tile(
                    [P, s_d], f32, name="psum_h")
                for ik in range(ni):
                    nc.tensor.matmul(
                        psum_h[:],
                        e_sb[:, ik, :],
                        qh[:, ik, st * s_d:(st + 1) * s_d],
                        start=(ik == 0), stop=(ik == ni - 1))
                nc.vector.tensor_scalar_mul(
                    h_sb[:, st * s_d:(st + 1) * s_d], psum_h[:], rS[:])
                nc.sync.dma_start(
                    out=h_d[b, jt * P:(jt + 1) * P, st * s_d:(st + 1) * s_d],
                    in_=h_sb[:, st * s_d:(st + 1) * s_d])

        def warmup(wu, n):
            # dummy matmuls with no input dependencies: fill the PE's
            # DMA-wait windows in the prologue so the DVFS/HAM busy streak
            # builds and the first real matmuls run at full clock.
            ps_w = ps_wu.tile([P, 512], f32, name="ps_w")
            for i in range(n):
                nc.tensor.matmul(ps_w[:], wu[:, 0:P], wu[:, 0:512],
                                 start=(i == 0), stop=(i == n - 1))

        def body():
            wu = const.tile([P, 512], fp16, name="wu")
            nc.vector.memset(wu[:], 0.0)
            warmup(wu, 12)
            # aT(0..2) first: they lead the SP queue (gemm1(0) needs aT(0)
            # before anything else), then the q loads fill in behind.
            aTs = {}
            for jt in range(3):
                aTs[(0, jt)] = a_prep(0, jt)
            start_q_loads(0, prologue=True)
            # pending depth 2: gemm2(t-2) runs after stats(t), so the first
            # gemm2's qh dependency moves out a full gemm-period (the
            # prologue is HBM-bandwidth-bound; qh arrives last).
            from collections import deque
            pending = deque()
            for b in range(bloc):
                qT_lo, qT_hi, qh = q_pend.pop(b)
                for jt in range(nj):
                    ps = ps_s.tile([P, nq], f32, name="ps_s")
                    gemm1(aTs.pop((b, jt)), (qT_lo, qT_hi), ps,
                          fill=(wu if b == 0 and jt < 2 else None))
                    e_sb, rS = stats(ps)
                    # rolling prefetches: next a tiles, next batch's q
                    nxt = (b, jt + 3) if jt + 3 < nj else (b + 1, jt + 3 - nj)
                    if nxt[0] < bloc:
                        aTs[nxt] = a_prep(*nxt)
                    if jt == 4:
                        start_q_loads(b + 1)
                    if len(pending) == 2:
                        p = pending.popleft()
                        gemm2(*p, chunked=(p[0] == 0 and p[1] == 0))
                    pending.append((b, jt, e_sb, rS, qh))
            gemm2(*pending.popleft())
            gemm2(*pending.popleft(), tail=True)

        if reps == 1:
            for _ in range(unroll):
                body()
        else:
            with tc.For_i(0, reps, 1):
                body()

    nc.compile()
    return nc


_CACHE = {}


def _get_program():
    key = "main"
    if key not in _CACHE:
        _CACHE[key] = build()
    return _CACHE[key]


def host_prep(q: np.ndarray, a: np.ndarray):
    """Cast to fp16 + pre-transpose into the DMA-perfect device layouts.
    Returns per-core input dicts (list of NCORES dicts)."""
    q = np.asarray(q, dtype=np.float32)
    a = np.asarray(a, dtype=np.float32)
    assert q.shape == (B, NQ, D) and a.shape == (B, NA, D), (q.shape, a.shape)
    q16 = q.astype(np.float16)
    a16 = a.astype(np.float16)
    # qh[b, p, ik, d] = q[b, ik*128+p, d]
    qh = np.ascontiguousarray(
        q16.reshape(B, NQ // P, P, D).transpose(0, 2, 1, 3))
    # qT[b, h, p, k, i] = q[b, h*1024+i, k*128+p]
    qT = np.ascontiguousarray(
        q16.reshape(B, 2, NQ // 2, D // P, P).transpose(0, 1, 4, 3, 2))
    # aT[b, jt, p, k, j] = a[b, jt*128+j, k*128+p]
    aT = np.ascontiguousarray(
        a16.reshape(B, NA // P, P, D // P, P).transpose(0, 1, 4, 3, 2))
    in_maps = []
    for c in range(NCORES):
        lo, hi = c * BLOC, (c + 1) * BLOC
        in_maps.append({"qh": qh[lo:hi], "qT": qT[lo:hi], "aT": aT[lo:hi]})
    return in_maps


def kernel(q: np.ndarray, a: np.ndarray) -> np.ndarray:
    from concourse import bass_utils

    nc = _get_program()
    in_maps = host_prep(q, a)
    res = bass_utils.run_bass_kernel_spmd(nc, in_maps, core_ids=list(range(NCORES)))
    out = np.concatenate(
        [np.asarray(res.results[c]["h"], dtype=np.float32)
         for c in range(NCORES)], axis=0)
    return out
